# revision 17
# baseline (speedup 1.0000x reference)
"""MoE MLP (8 experts, top-2) Trainium2 kernel.

Strategy (expert-parallel, per the sharding hint):
  - Router (logits/softmax/top-k/aux-loss) is computed host-side with jax on
    CPU, using the exact op sequence of the reference so routing decisions and
    aux_loss match bit-for-bit.
  - Tokens are dispatched (gathered) host-side per expert; expert e's tokens
    go to core e, padded to a common capacity C.
  - Each core runs a dense fused MLP for its expert in bf16 (fp32 PSUM
    accumulation): gu^T = w_gu^T @ x^T, h^T = silu(g^T)*u^T, y^T = w_dn^T @ h^T.
    All activations live transposed (tokens on the free dim) so every matmul
    uses natural weight layouts and no on-chip transposes are needed.
  - The routed combine (weighted scatter-add) happens host-side during the
    unshard step.
"""

import numpy as np
import ml_dtypes

H = 1024
I = 2816
TWO_I = 2 * I
E = 8
P = 128
KH = H // P    # 8  k-tiles for the up projection
KI = I // P    # 22 k-tiles for the down projection
MJ = I // P    # 22 gate/up column-pair chunks
MH = H // P    # 8  output row chunks
NB = 512       # token block (matmul free dim)

BF16 = ml_dtypes.bfloat16

# test harness hooks
TRACE = False
TRACE_CORES = None
LAST_RESULTS = None

_nc_cache = {}


def _ensure_ntff_hook_importable():
    """bass_utils' axon trace path imports antenv.axon_hooks, which this
    image lacks. Install a shim (ctypes into libaxon_pjrt.so) so tracing
    works — and so a stray BASS_TRACE=1 in the environment can't crash an
    untraced run with ModuleNotFoundError."""
    import sys
    try:
        import antenv.axon_hooks  # noqa: F401
        return
    except ImportError:
        pass
    import contextlib
    import ctypes
    import types

    def _make_hook():
        try:
            lib = ctypes.CDLL("/opt/axon/libaxon_pjrt.so")
        except OSError:
            return None
        if not hasattr(lib, "axon_start_nrt_profile"):
            return None
        lib.axon_start_nrt_profile.argtypes = [
            ctypes.POINTER(ctypes.c_int64), ctypes.c_size_t]
        lib.axon_start_nrt_profile.restype = ctypes.c_int64
        lib.axon_stop_nrt_profile.argtypes = [ctypes.c_char_p]
        lib.axon_stop_nrt_profile.restype = ctypes.c_int64

        @contextlib.contextmanager
        def _hook(output_dir, device_ids):
            import jax
            jax.devices()
            if device_ids:
                ids = (ctypes.c_int64 * len(device_ids))(*device_ids)
                rc = lib.axon_start_nrt_profile(ids, len(device_ids))
            else:
                rc = lib.axon_start_nrt_profile(None, 0)
            if rc != 0:
                raise RuntimeError(f"axon_start_nrt_profile rc={rc}")
            try:
                yield
            finally:
                lib.axon_stop_nrt_profile(str(output_dir).encode())

        return _hook

    hook = _make_hook()
    mod = types.ModuleType("antenv.axon_hooks")
    mod.get_axon_ntff_profile_hook = lambda: hook
    mod.set_axon_ntff_profile_hook = lambda h: None
    sys.modules["antenv.axon_hooks"] = mod


def _build_nc(C):
    """Build + compile the per-core Bass program for capacity C (multiple of 128)."""
    import concourse.bass as bass  # noqa: F401
    import concourse.mybir as mybir
    import concourse.tile as tile
    from concourse import bacc

    dt = mybir.dt
    nc = bacc.Bacc("TRN2", target_bir_lowering=False, debug=False)

    xT = nc.dram_tensor("xT", (H, C), dt.bfloat16, kind="ExternalInput").ap()
    wgu = nc.dram_tensor("wgu", (H, TWO_I), dt.bfloat16, kind="ExternalInput").ap()
    wdn = nc.dram_tensor("wdn", (I, H), dt.bfloat16, kind="ExternalInput").ap()
    yT = nc.dram_tensor("yT", (H, C), dt.float32, kind="ExternalOutput").ap()

    xT_r = xT.rearrange("(k p) c -> p k c", p=P)       # [P, KH, C]
    yT_r = yT.rearrange("(m p) c -> m p c", p=P)       # [MH, P, C]

    # Near-equal block widths (all >=264 when possible) so every matmul's
    # free dim is large enough to hide LDWEIGHTS (needs N >= ~257; a 256-wide
    # block measured ~14us slower end-to-end than the even split).
    # Smallest block first so the initial x DMA lands sooner.
    nblk = max(1, -(-C // NB))
    base, rem = divmod(C, nblk)
    widths = sorted([base + 1] * rem + [base] * (nblk - rem))
    blocks = []
    n0 = 0
    for w in widths:
        blocks.append((n0, w))
        n0 += w

    with tile.TileContext(nc) as tc:
        with (
            tc.tile_pool(name="wgu_p", bufs=1) as wgu_pool,
            tc.tile_pool(name="wdn_p", bufs=1) as wdn_pool,
            tc.tile_pool(name="xt_p", bufs=2) as xt_pool,
            tc.tile_pool(name="ht_p", bufs=1) as ht_pool,
            tc.tile_pool(name="tmp_p", bufs=3) as tmp_pool,
            tc.tile_pool(name="y_p", bufs=3) as y_pool,
            tc.tile_pool(name="ps_gu", bufs=3, space="PSUM") as psum_gu,
            tc.tile_pool(name="ps_y", bufs=2, space="PSUM") as psum_y,
        ):
            # First token block goes out first so the PE can start ASAP.
            xt0 = xt_pool.tile([P, KH, NB], dt.bfloat16, tag="xt", name="xt")
            nc.sync.dma_start(out=xt0[:, :, :blocks[0][1]],
                              in_=xT_r[:, :, :blocks[0][1]])

            # Resident weights (bf16): 8*11KB + 22*2KB = 132KB/partition,
            # each held in ONE big tile so a single dma_start moves a large
            # chunk (the per-dma_start trigger costs ~0.6us of sequencer time,
            # so few large DMAs keep the weight stream ahead of the PE).
            # w_gu streams in j-ordered column chunks (g and u ranges paired
            # with the stage-1 consumption order), small chunks first to
            # minimize time-to-first-matmul.
            # Trigger weight DMAs from otherwise-idle engines: each dma_start
            # costs ~0.6-2us of sequencer time, and serializing them all on
            # Sync (behind xt0) delays the first matmul by several us.
            wgu_b = wgu_pool.tile([P, KH, TWO_I], dt.bfloat16, tag="wgu", name="wgu")
            wgu_pkc = wgu.rearrange("(k p) n -> p k n", p=P)   # [P, KH, 2I]
            j0 = 0
            for ci, jn in enumerate((1, 1, 2, 2, 4, 4, 4, 4)):
                c0, c1 = j0 * P, (j0 + jn) * P
                g_eng = nc.scalar if ci < 4 else nc.sync
                g_eng.dma_start(out=wgu_b[:, :, c0:c1],
                                in_=wgu_pkc[:, :, c0:c1])
                nc.gpsimd.dma_start(out=wgu_b[:, :, I + c0:I + c1],
                                    in_=wgu_pkc[:, :, I + c0:I + c1])
                j0 += jn
            wdn_b = wdn_pool.tile([P, KI, H], dt.bfloat16, tag="wdn", name="wdn")
            wdn_pkc = wdn.rearrange("(k p) n -> p k n", p=P)   # [P, KI, H]
            nc.gpsimd.dma_start(out=wdn_b, in_=wdn_pkc)

            for bi, (n0, nb) in enumerate(blocks):
                if bi == 0:
                    xt = xt0
                else:
                    xt = xt_pool.tile([P, KH, NB], dt.bfloat16, tag="xt", name="xt")
                    nc.sync.dma_start(out=xt[:, :, :nb],
                                      in_=xT_r[:, :, n0:n0 + nb])

                # stage 1: gu^T = w_gu^T @ x^T, fused silu(g)*u -> h^T (bf16)
                ht_t = []
                for j in range(MJ):
                    pg = psum_gu.tile([P, NB], dt.float32, tag="pg", name="pg")
                    pu = psum_gu.tile([P, NB], dt.float32, tag="pu", name="pu")
                    for k in range(KH):
                        nc.tensor.matmul(
                            pg[:, :nb],
                            lhsT=wgu_b[:, k, j * P:(j + 1) * P],
                            rhs=xt[:, k, :nb],
                            start=(k == 0), stop=(k == KH - 1),
                        )
                    for k in range(KH):
                        nc.tensor.matmul(
                            pu[:, :nb],
                            lhsT=wgu_b[:, k, (MJ + j) * P:(MJ + j + 1) * P],
                            rhs=xt[:, k, :nb],
                            start=(k == 0), stop=(k == KH - 1),
                        )
                    sg = tmp_pool.tile([P, NB], dt.float32, tag="sg", name="sg")
                    nc.scalar.activation(sg[:, :nb], pg[:, :nb],
                                         mybir.ActivationFunctionType.Sigmoid)
                    t1 = tmp_pool.tile([P, NB], dt.float32, tag="t1", name="t1")
                    nc.vector.tensor_mul(t1[:, :nb], sg[:, :nb], pg[:, :nb])
                    ht = ht_pool.tile([P, NB], dt.bfloat16, tag=f"ht{j}", name=f"ht{j}")
                    nc.vector.tensor_mul(ht[:, :nb], t1[:, :nb], pu[:, :nb])
                    ht_t.append(ht)

                # stage 2: y^T = w_dn^T @ h^T
                for m in range(MH):
                    py = psum_y.tile([P, NB], dt.float32, tag="py", name="py")
                    for k in range(KI):
                        nc.tensor.matmul(
                            py[:, :nb],
                            lhsT=wdn_b[:, k, m * P:(m + 1) * P],
                            rhs=ht_t[k][:, :nb],
                            start=(k == 0), stop=(k == KI - 1),
                        )
                    ysb = y_pool.tile([P, NB], dt.float32, tag="ysb", name="ysb")
                    nc.scalar.copy(ysb[:, :nb], py[:, :nb])
                    nc.sync.dma_start(out=yT_r[m][:, n0:n0 + nb], in_=ysb[:, :nb])

    nc.compile()
    return nc


def _get_nc(C):
    nc = _nc_cache.get(C)
    if nc is None:
        nc = _build_nc(C)
        _nc_cache[C] = nc
    return nc


def kernel(x, gate_w, w_gate_up, w_down, top_k):
    global LAST_RESULTS
    import jax
    import jax.numpy as jnp
    from concourse.bass_utils import run_bass_kernel_spmd

    K = int(top_k)
    x = np.asarray(x, dtype=np.float32)
    gate_w = np.asarray(gate_w, dtype=np.float32)
    w_gate_up = np.asarray(w_gate_up, dtype=np.float32)
    w_down = np.asarray(w_down, dtype=np.float32)
    B, S, Hx = x.shape
    T = B * S

    # ---- Router on host CPU, replicating the reference op-for-op ----
    cpu = jax.devices("cpu")[0]
    with jax.default_device(cpu):
        xj = jnp.asarray(x)
        logits = jnp.einsum("bsh,eh->bse", xj, jnp.asarray(gate_w))
        probs = jax.nn.softmax(logits.astype(jnp.float32), axis=-1)
        topk_vals, topk_idx = jax.lax.top_k(probs, K)
        topk_w = topk_vals / jnp.clip(topk_vals.sum(-1, keepdims=True), 1e-8)
        importance = probs.mean((0, 1))
        onehot = jax.nn.one_hot(topk_idx, E, dtype=probs.dtype)
        sel_mask = onehot.sum(-2)
        load = sel_mask.sum((0, 1)) / (T * K)
        aux_loss = (E * (importance * load)).sum()
    aux_loss = np.asarray(aux_loss)
    ti = np.asarray(topk_idx).reshape(T, K)
    tw = np.asarray(topk_w).reshape(T, K).astype(np.float32)

    # ---- Dispatch: gather tokens per expert, pad to capacity ----
    xf = x.reshape(T, Hx)
    ids, wts = [], []
    for e in range(E):
        sel = ti == e                       # [T, K]
        rows = np.nonzero(sel.any(-1))[0]
        slot = sel[rows].argmax(-1)
        ids.append(rows)
        wts.append(tw[rows, slot])
    counts = [len(r) for r in ids]
    C = max(NB, -(-max(counts) // P) * P)

    in_maps = []
    for e in range(E):
        xT_e = np.zeros((Hx, C), dtype=BF16)
        xT_e[:, :counts[e]] = xf[ids[e]].T
        in_maps.append({
            "xT": xT_e,
            "wgu": np.ascontiguousarray(w_gate_up[e]).astype(BF16),
            "wdn": np.ascontiguousarray(w_down[e]).astype(BF16),
        })

    _ensure_ntff_hook_importable()
    nc = _get_nc(C)
    res = run_bass_kernel_spmd(nc, in_maps, core_ids=list(range(E)), trace=TRACE,
                               trace_cores=TRACE_CORES)
    LAST_RESULTS = res

    # ---- Combine: weighted scatter-add (unshard) ----
    out = np.zeros((T, Hx), dtype=np.float32)
    for e in range(E):
        n_e = counts[e]
        yT_e = res.results[e]["yT"]          # [H, C] fp32
        out[ids[e]] += yT_e[:, :n_e].T * wts[e][:, None]

    return out.reshape(B, S, Hx), aux_loss


# revision 18
# speedup vs baseline: 1.0769x; 1.0769x over previous
"""MoE MLP (8 experts, top-2) Trainium2 kernel.

Strategy (expert-parallel, per the sharding hint):
  - Router (logits/softmax/top-k/aux-loss) is computed host-side with jax on
    CPU, using the exact op sequence of the reference so routing decisions and
    aux_loss match bit-for-bit.
  - Tokens are dispatched (gathered) host-side per expert; expert e's tokens
    go to core e, padded to a common capacity C.
  - Each core runs a dense fused MLP for its expert in bf16 (fp32 PSUM
    accumulation): gu^T = w_gu^T @ x^T, h^T = silu(g^T)*u^T, y^T = w_dn^T @ h^T.
    All activations live transposed (tokens on the free dim) so every matmul
    uses natural weight layouts and no on-chip transposes are needed.
  - The routed combine (weighted scatter-add) happens host-side during the
    unshard step.
"""

import numpy as np
import ml_dtypes

H = 1024
I = 2816
TWO_I = 2 * I
E = 8
P = 128
KH = H // P    # 8  k-tiles for the up projection
KI = I // P    # 22 k-tiles for the down projection
MJ = I // P    # 22 gate/up column-pair chunks
MH = H // P    # 8  output row chunks
NB = 512       # token block (matmul free dim)

BF16 = ml_dtypes.bfloat16

# test harness hooks
TRACE = False
TRACE_CORES = None
LAST_RESULTS = None

_nc_cache = {}


def _ensure_ntff_hook_importable():
    """bass_utils' axon trace path imports antenv.axon_hooks, which this
    image lacks. Install a shim (ctypes into libaxon_pjrt.so) so tracing
    works — and so a stray BASS_TRACE=1 in the environment can't crash an
    untraced run with ModuleNotFoundError."""
    import sys
    try:
        import antenv.axon_hooks  # noqa: F401
        return
    except ImportError:
        pass
    import contextlib
    import ctypes
    import types

    def _make_hook():
        try:
            lib = ctypes.CDLL("/opt/axon/libaxon_pjrt.so")
        except OSError:
            return None
        if not hasattr(lib, "axon_start_nrt_profile"):
            return None
        lib.axon_start_nrt_profile.argtypes = [
            ctypes.POINTER(ctypes.c_int64), ctypes.c_size_t]
        lib.axon_start_nrt_profile.restype = ctypes.c_int64
        lib.axon_stop_nrt_profile.argtypes = [ctypes.c_char_p]
        lib.axon_stop_nrt_profile.restype = ctypes.c_int64

        @contextlib.contextmanager
        def _hook(output_dir, device_ids):
            import jax
            jax.devices()
            if device_ids:
                ids = (ctypes.c_int64 * len(device_ids))(*device_ids)
                rc = lib.axon_start_nrt_profile(ids, len(device_ids))
            else:
                rc = lib.axon_start_nrt_profile(None, 0)
            if rc != 0:
                raise RuntimeError(f"axon_start_nrt_profile rc={rc}")
            try:
                yield
            finally:
                lib.axon_stop_nrt_profile(str(output_dir).encode())

        return _hook

    hook = _make_hook()
    mod = types.ModuleType("antenv.axon_hooks")
    mod.get_axon_ntff_profile_hook = lambda: hook
    mod.set_axon_ntff_profile_hook = lambda h: None
    sys.modules["antenv.axon_hooks"] = mod


def _build_nc(C):
    """Build + compile the per-core Bass program for capacity C (multiple of 128)."""
    import concourse.bass as bass  # noqa: F401
    import concourse.mybir as mybir
    import concourse.tile as tile
    from concourse import bacc

    dt = mybir.dt
    nc = bacc.Bacc("TRN2", target_bir_lowering=False, debug=False)

    xT = nc.dram_tensor("xT", (H, C), dt.bfloat16, kind="ExternalInput").ap()
    wgu = nc.dram_tensor("wgu", (H, TWO_I), dt.bfloat16, kind="ExternalInput").ap()
    wdn = nc.dram_tensor("wdn", (I, H), dt.bfloat16, kind="ExternalInput").ap()
    yT = nc.dram_tensor("yT", (H, C), dt.float32, kind="ExternalOutput").ap()

    xT_r = xT.rearrange("(k p) c -> p k c", p=P)       # [P, KH, C]
    yT_r = yT.rearrange("(m p) c -> m p c", p=P)       # [MH, P, C]

    # Near-equal block widths (all >=264 when possible) so every matmul's
    # free dim is large enough to hide LDWEIGHTS (needs N >= ~257; a 256-wide
    # block measured ~14us slower end-to-end than the even split).
    # Smallest block first so the initial x DMA lands sooner.
    nblk = max(1, -(-C // NB))
    base, rem = divmod(C, nblk)
    widths = sorted([base + 1] * rem + [base] * (nblk - rem))
    blocks = []
    n0 = 0
    for w in widths:
        blocks.append((n0, w))
        n0 += w

    with tile.TileContext(nc) as tc:
        with (
            tc.tile_pool(name="wgu_p", bufs=1) as wgu_pool,
            tc.tile_pool(name="wdn_p", bufs=1) as wdn_pool,
            tc.tile_pool(name="xt_p", bufs=2) as xt_pool,
            tc.tile_pool(name="ht_p", bufs=1) as ht_pool,
            tc.tile_pool(name="tmp_p", bufs=3) as tmp_pool,
            tc.tile_pool(name="y_p", bufs=3) as y_pool,
            tc.tile_pool(name="ps_gu", bufs=3, space="PSUM") as psum_gu,
            tc.tile_pool(name="ps_y", bufs=2, space="PSUM") as psum_y,
        ):
            # First token block goes out first so the PE can start ASAP.
            xt0 = xt_pool.tile([P, KH, NB], dt.bfloat16, tag="xt", name="xt")
            nc.sync.dma_start(out=xt0[:, :, :blocks[0][1]],
                              in_=xT_r[:, :, :blocks[0][1]])

            # Resident weights (bf16): 8*11KB + 22*2KB = 132KB/partition,
            # each held in ONE big tile so a single dma_start moves a large
            # chunk (the per-dma_start trigger costs ~0.6us of sequencer time,
            # so few large DMAs keep the weight stream ahead of the PE).
            # w_gu streams in j-ordered column chunks (g and u ranges paired
            # with the stage-1 consumption order), small chunks first to
            # minimize time-to-first-matmul.
            # Trigger weight DMAs from otherwise-idle engines: each dma_start
            # costs ~0.6-2us of sequencer time, and serializing them all on
            # Sync (behind xt0) delays the first matmul by several us.
            wgu_b = wgu_pool.tile([P, KH, TWO_I], dt.bfloat16, tag="wgu", name="wgu")
            wgu_pkc = wgu.rearrange("(k p) n -> p k n", p=P)   # [P, KH, 2I]
            j0 = 0
            for jn in (1, 1, 2, 2, 4, 4, 4, 4):
                c0, c1 = j0 * P, (j0 + jn) * P
                nc.sync.dma_start(out=wgu_b[:, :, c0:c1],
                                  in_=wgu_pkc[:, :, c0:c1])
                nc.sync.dma_start(out=wgu_b[:, :, I + c0:I + c1],
                                  in_=wgu_pkc[:, :, I + c0:I + c1])
                j0 += jn
            wdn_b = wdn_pool.tile([P, KI, H], dt.bfloat16, tag="wdn", name="wdn")
            wdn_pkc = wdn.rearrange("(k p) n -> p k n", p=P)   # [P, KI, H]
            nc.sync.dma_start(out=wdn_b, in_=wdn_pkc)

            for bi, (n0, nb) in enumerate(blocks):
                if bi == 0:
                    xt = xt0
                else:
                    xt = xt_pool.tile([P, KH, NB], dt.bfloat16, tag="xt", name="xt")
                    nc.sync.dma_start(out=xt[:, :, :nb],
                                      in_=xT_r[:, :, n0:n0 + nb])

                # stage 1: gu^T = w_gu^T @ x^T, fused silu(g)*u -> h^T (bf16)
                ht_t = []
                for j in range(MJ):
                    pg = psum_gu.tile([P, NB], dt.float32, tag="pg", name="pg")
                    pu = psum_gu.tile([P, NB], dt.float32, tag="pu", name="pu")
                    for k in range(KH):
                        nc.tensor.matmul(
                            pg[:, :nb],
                            lhsT=wgu_b[:, k, j * P:(j + 1) * P],
                            rhs=xt[:, k, :nb],
                            start=(k == 0), stop=(k == KH - 1),
                        )
                    for k in range(KH):
                        nc.tensor.matmul(
                            pu[:, :nb],
                            lhsT=wgu_b[:, k, (MJ + j) * P:(MJ + j + 1) * P],
                            rhs=xt[:, k, :nb],
                            start=(k == 0), stop=(k == KH - 1),
                        )
                    sg = tmp_pool.tile([P, NB], dt.float32, tag="sg", name="sg")
                    nc.scalar.activation(sg[:, :nb], pg[:, :nb],
                                         mybir.ActivationFunctionType.Sigmoid)
                    t1 = tmp_pool.tile([P, NB], dt.float32, tag="t1", name="t1")
                    nc.vector.tensor_mul(t1[:, :nb], sg[:, :nb], pg[:, :nb])
                    ht = ht_pool.tile([P, NB], dt.bfloat16, tag=f"ht{j}", name=f"ht{j}")
                    nc.vector.tensor_mul(ht[:, :nb], t1[:, :nb], pu[:, :nb])
                    ht_t.append(ht)

                # stage 2: y^T = w_dn^T @ h^T
                for m in range(MH):
                    py = psum_y.tile([P, NB], dt.float32, tag="py", name="py")
                    for k in range(KI):
                        nc.tensor.matmul(
                            py[:, :nb],
                            lhsT=wdn_b[:, k, m * P:(m + 1) * P],
                            rhs=ht_t[k][:, :nb],
                            start=(k == 0), stop=(k == KI - 1),
                        )
                    ysb = y_pool.tile([P, NB], dt.float32, tag="ysb", name="ysb")
                    nc.scalar.copy(ysb[:, :nb], py[:, :nb])
                    nc.sync.dma_start(out=yT_r[m][:, n0:n0 + nb], in_=ysb[:, :nb])

    nc.compile()
    return nc


def _get_nc(C):
    nc = _nc_cache.get(C)
    if nc is None:
        nc = _build_nc(C)
        _nc_cache[C] = nc
    return nc


def kernel(x, gate_w, w_gate_up, w_down, top_k):
    global LAST_RESULTS
    import jax
    import jax.numpy as jnp
    from concourse.bass_utils import run_bass_kernel_spmd

    K = int(top_k)
    x = np.asarray(x, dtype=np.float32)
    gate_w = np.asarray(gate_w, dtype=np.float32)
    w_gate_up = np.asarray(w_gate_up, dtype=np.float32)
    w_down = np.asarray(w_down, dtype=np.float32)
    B, S, Hx = x.shape
    T = B * S

    # ---- Router on host CPU, replicating the reference op-for-op ----
    cpu = jax.devices("cpu")[0]
    with jax.default_device(cpu):
        xj = jnp.asarray(x)
        logits = jnp.einsum("bsh,eh->bse", xj, jnp.asarray(gate_w))
        probs = jax.nn.softmax(logits.astype(jnp.float32), axis=-1)
        topk_vals, topk_idx = jax.lax.top_k(probs, K)
        topk_w = topk_vals / jnp.clip(topk_vals.sum(-1, keepdims=True), 1e-8)
        importance = probs.mean((0, 1))
        onehot = jax.nn.one_hot(topk_idx, E, dtype=probs.dtype)
        sel_mask = onehot.sum(-2)
        load = sel_mask.sum((0, 1)) / (T * K)
        aux_loss = (E * (importance * load)).sum()
    aux_loss = np.asarray(aux_loss)
    ti = np.asarray(topk_idx).reshape(T, K)
    tw = np.asarray(topk_w).reshape(T, K).astype(np.float32)

    # ---- Dispatch: gather tokens per expert, pad to capacity ----
    xf = x.reshape(T, Hx)
    ids, wts = [], []
    for e in range(E):
        sel = ti == e                       # [T, K]
        rows = np.nonzero(sel.any(-1))[0]
        slot = sel[rows].argmax(-1)
        ids.append(rows)
        wts.append(tw[rows, slot])
    counts = [len(r) for r in ids]
    C = max(NB, -(-max(counts) // P) * P)

    in_maps = []
    for e in range(E):
        xT_e = np.zeros((Hx, C), dtype=BF16)
        xT_e[:, :counts[e]] = xf[ids[e]].T
        in_maps.append({
            "xT": xT_e,
            "wgu": np.ascontiguousarray(w_gate_up[e]).astype(BF16),
            "wdn": np.ascontiguousarray(w_down[e]).astype(BF16),
        })

    _ensure_ntff_hook_importable()
    nc = _get_nc(C)
    res = run_bass_kernel_spmd(nc, in_maps, core_ids=list(range(E)), trace=TRACE,
                               trace_cores=TRACE_CORES)
    LAST_RESULTS = res

    # ---- Combine: weighted scatter-add (unshard) ----
    out = np.zeros((T, Hx), dtype=np.float32)
    for e in range(E):
        n_e = counts[e]
        yT_e = res.results[e]["yT"]          # [H, C] fp32
        out[ids[e]] += yT_e[:, :n_e].T * wts[e][:, None]

    return out.reshape(B, S, Hx), aux_loss
